# revision 40
# baseline (speedup 1.0000x reference)
"""MultiHeadAttention TRN2 kernel: B=2, L=2048, DIM=1024, 16 heads x 64.

Sharding: 8 cores = 2 (batch) x 4 (head groups of 4 heads), tensor-parallel
on heads (Wq/Wk/Wv column-split) with Wo ROW-split: each core computes a
full-width partial output out_partial[L, DIM] from its 4 heads; the host
sums the 4 partials per batch during unshard (the all-reduce of the
row-split Wo, performed at gather time).  No device collectives.

Per core (all matmul operands fp16, PSUM accumulation fp32):
  - xT16 = q[b].T [1024, 2048], wq/wk/wv = W.T[:, headslice] [1024, 256],
    wo = Wo.T[headslice, :] [256, 1024] -- all fp16, host-converted.
  - K projected first (transposed layout KT [d, j]), then Q chunk-0, so
    scores/exp for chunk 0 start ~20us in; V (natural [j, h, 64+1] with a
    ones column -> softmax denominator) and remaining Q quarters follow.
  - scores per (chunk, head): j-tile pairs share one PSUM tile so one
    ACTIVATE exps 1024 elems; exp(0.125*s) -> fp16 attn tiles [j, i].
  - AV in NATURAL orientation (lhsT=attnT, rhs=[v|1]): out [i-block, 65]
    -- full 128-row stationary vs 65 in the transposed form (2x fewer PE
    rows), denominator lands in column 64 per i-PARTITION, so the
    normalize is a per-partition tensor_scalar (no PE broadcast matmul).
  - head pairs packed side by side [i, 128], DMA-TRANSPOSED (xbar, off-PE)
    into the Wo lhsT layout [2x64 d, i].
  - Wo partial: out[i, 1024] = sum over 2 d-pair k-tiles; PSUM -> SBUF f32
    copy on gpsimd (Pool), DMA out.  Host sums group partials.
Emission interleaves scores(c+1, h) ahead of AV(c, h) per head so the PE
stays busy while ACT (the #2 engine, ~128us of exp) catches up.
"""

import sys
from contextlib import ExitStack

import numpy as np

for _p in ("/opt/trn_rl_repo",):
    if _p not in sys.path:
        sys.path.insert(0, _p)

import concourse.bass as bass
import concourse.tile as tile
from concourse import bacc, masks, mybir
from concourse.bass_utils import run_bass_kernel_spmd

F32 = mybir.dt.float32
F16 = mybir.dt.float16

B, L, DIM = 2, 2048, 1024
NH, HD = 16, 64           # total heads, head dim
HL = 4                    # heads per core
DL = HL * HD              # local head dims = 256
KT = DIM // 128           # 8  contraction k-tiles
JT = L // 128             # 16 j (key) tiles
CH = 512                  # i-chunk size
NCH = L // CH             # 4 chunks
NQ = 4                    # L quarters for projection streaming
QLF = L // NQ             # 512


def build_nc():
    nc = bacc.Bacc("TRN2", target_bir_lowering=False, debug=False, num_devices=8)

    xT_d = nc.dram_tensor("xT", [DIM, L], F16, kind="ExternalInput")
    wq_d = nc.dram_tensor("wq", [DIM, DL], F16, kind="ExternalInput")
    wk_d = nc.dram_tensor("wk", [DIM, DL], F16, kind="ExternalInput")
    wv_d = nc.dram_tensor("wv", [DIM, DL], F16, kind="ExternalInput")
    wo_d = nc.dram_tensor("wo", [DL, DIM], F16, kind="ExternalInput")
    out_d = nc.dram_tensor("out", [L, DIM], F32, kind="ExternalOutput")

    with tile.TileContext(nc) as tc:
        with ExitStack() as ctx:
            wpool = ctx.enter_context(tc.tile_pool(name="weights", bufs=3))
            wopool = ctx.enter_context(tc.tile_pool(name="wo", bufs=2))
            xqpool = ctx.enter_context(tc.tile_pool(name="xTq", bufs=4))
            qkpool = ctx.enter_context(tc.tile_pool(name="qk", bufs=16))
            vpool = ctx.enter_context(tc.tile_pool(name="v", bufs=16))
            atpool = ctx.enter_context(tc.tile_pool(name="attnT", bufs=40))
            aopool = ctx.enter_context(tc.tile_pool(name="ao", bufs=10))
            aotpool = ctx.enter_context(tc.tile_pool(name="aot", bufs=18))
            small = ctx.enter_context(tc.tile_pool(name="small", bufs=6))
            outpool = ctx.enter_context(tc.tile_pool(name="outsb", bufs=4))
            # ps_s: 3 slots of [128, 1024] f32 (2 banks each): ~2.7us of
            # ACT lookahead with fine production granularity.
            ps_s = ctx.enter_context(
                tc.tile_pool(name="ps_s", bufs=3, space="PSUM"))
            # ps_mix: everything else (proj [128,512], AV [128,65],
            # PE-transpose [128,128]f16, Wo [128,512]) in 2 one-bank slots
            ps_mix = ctx.enter_context(
                tc.tile_pool(name="ps_mix", bufs=2, space="PSUM"))

            # ---- weights (as [128, KT, DL] k-tile stacks) ----
            # HWDGE (SP/ACT-issued DMA) is an exclusive device with a 625ns
            # hold per DMA: split loads between SP (HWDGE) and gpsimd (SWDGE,
            # desc-gen on the otherwise idle Pool engine) so neither serializes
            # the projection pipeline.
            def load_w(dram_t, name, eng):
                t = wpool.tile([128, KT, DL], F16, name=name, tag="w")
                eng.dma_start(
                    out=t[:], in_=dram_t[:].rearrange("(k p) n -> p k n", p=128))
                return t

            # wk split in k-halves across SP (HWDGE) and gpsimd (SWDGE):
            # the first half lands ~1.3us earlier, so K proj starts sooner
            wk_view = wk_d[:].rearrange("(k p) n -> p k n", p=128)
            wk_lo = wpool.tile([128, KT // 2, DL], F16, name="wk_lo", tag="wkh")
            nc.sync.dma_start(out=wk_lo[:], in_=wk_view[:, 0:KT // 2, :])
            wk_hi = wpool.tile([128, KT // 2, DL], F16, name="wk_hi", tag="wkh")
            nc.gpsimd.dma_start(out=wk_hi[:], in_=wk_view[:, KT // 2:KT, :])

            # xT loads, all SP/HWDGE: quarter 0 split per-k (first tile lands
            # ~2us so K proj starts immediately); quarters 1-3 as ONE big DMA
            # each (one 625ns HWDGE hold instead of eight)
            xT_view = xT_d[:].rearrange("(k p) n -> p k n", p=128)
            xT_k = [[None] * KT for _ in range(NQ)]

            def load_xtq(qi):
                xt = xqpool.tile([128, KT, QLF], F16, name="xtq", tag="xtq")
                nc.sync.dma_start(
                    out=xt[:], in_=xT_view[:, :, qi * QLF:(qi + 1) * QLF])
                for k in range(KT):
                    xT_k[qi][k] = xt[:, k, :]

            wv_view = wv_d[:].rearrange("(k p) n -> p k n", p=128)

            def load_wv_half(t, eng):
                w = wpool.tile([128, KT, DL // 2], F16, name=f"wv{t}", tag="wvh")
                eng.dma_start(out=w[:], in_=wv_view[:, :, t * 128:(t + 1) * 128])
                return w

            # DMA_ENGINES is exclusive; order transfers by when compute
            # needs them: xt q0, wq, xt q1, wv(h01), xt q2, xt q3 on SP and
            # wk_hi / wv(h23) / wo riding the parallel SWDGE path.
            load_xtq(0)
            wq_sb = load_w(wq_d, "wq_sb", nc.sync)
            load_xtq(1)
            wv_lo = load_wv_half(0, nc.sync)
            load_xtq(2)
            load_xtq(3)
            wv_hi = load_wv_half(1, nc.gpsimd)
            wv_half = (wv_lo, wv_hi)
            # wo natural [256, 1024] -> 2 k-tiles [128, 1024]
            wo_view = wo_d[:].rearrange("(t p) n -> t p n", p=128)
            wo_sb = []
            for t in range(2):
                w = wopool.tile([128, DIM], F16, name=f"wo{t}", tag="wo")
                nc.gpsimd.dma_start(out=w[:], in_=wo_view[t])
                wo_sb.append(w)

            # fp16 identity for PE-transpose (53ns/tile vs 625ns HWDGE hold
            # for the xbar DMA transpose)
            ident = small.tile([128, 128], F16, name="ident", tag="ident")
            masks.make_identity(nc, ident[:])

            # V natural, one tile per (head, j-quarter): [128, 4, 65]
            # (col 64 = ones -> denominator).  Per-head tiles let the V
            # projection stream per head inside chunk 0's loop; batching 4
            # j-tiles per PSUM drain keeps the fill/drain ratio healthy.
            v_aug = [[vpool.tile([128, 4, HD + 1], F16, name="va", tag="va")
                      for _ in range(NQ)] for _ in range(HL)]
            for row in v_aug:
                for va in row:
                    nc.vector.memset(va[:, :, HD:HD + 1], 1.0)

            # ---- projections ----
            # per-quarter tiles keep Tile deps fine-grained
            QT = [[None] * NQ for _ in range(2)]
            KTt = [[None] * NQ for _ in range(2)]

            def qk_proj_n(w_sb, tiles, qi, n):
                ps = ps_mix.tile([128, QLF], F32, name="ps_p", tag="mix")
                for k in range(KT):
                    if isinstance(w_sb, tuple):
                        w = w_sb[k // (KT // 2)]
                        wsl = w[:, k % (KT // 2), n * 128:(n + 1) * 128]
                    else:
                        wsl = w_sb[:, k, n * 128:(n + 1) * 128]
                    nc.tensor.matmul(
                        ps[:], lhsT=wsl, rhs=xT_k[qi][k][:],
                        start=(k == 0), stop=(k == KT - 1))
                t = qkpool.tile([128, QLF], F16, name="qkt", tag="qkt")
                nc.vector.tensor_copy(out=t[:], in_=ps[:])
                tiles[n][qi] = t

            def qk_proj_quarter(w_sb, tiles, qi):
                for n in range(2):
                    qk_proj_n(w_sb, tiles, qi, n)

            def v_proj_hq(h, qi):
                ps = ps_mix.tile([128, 4, HD], F32, name="ps_v", tag="mix")
                wvh = wv_half[h // 2]
                for m in range(4):
                    for k in range(KT):
                        nc.tensor.matmul(
                            ps[:, m, :],
                            lhsT=xT_k[qi][k][:, m * 128:(m + 1) * 128],
                            rhs=wvh[:, k, (h % 2) * HD:(h % 2 + 1) * HD],
                            start=(k == 0), stop=(k == KT - 1))
                nc.vector.tensor_copy(
                    out=v_aug[h][qi][:, :, 0:HD], in_=ps[:])

            # ---- attention ----
            # j-tile pairs: one [128, 1024] exp per pair
            JG = [(2 * g, 2 * g + 1) for g in range(JT // 2)]

            def scores_group(c, h, grp):
                """one j-group's scores + exp -> fp16 attn tile
                [128 j, len(grp)*CH]."""
                ht, hr = h // 2, 64 * (h % 2)
                w = len(grp) * CH
                ps_sc = ps_s.tile([128, w], F32, name="ps_sc", tag="ps_s")
                for s, j in enumerate(grp):
                    nc.tensor.matmul(
                        ps_sc[:, s * CH:(s + 1) * CH],
                        lhsT=KTt[ht][j // 4][hr:hr + 64,
                                             (j % 4) * 128:(j % 4 + 1) * 128],
                        rhs=QT[ht][c][hr:hr + 64, :],
                        start=True, stop=True)
                at = atpool.tile([128, w], F16, name="at", tag="at")
                nc.scalar.activation(
                    out=at[:], in_=ps_sc[:],
                    func=mybir.ActivationFunctionType.Exp,
                    scale=1.0 / np.sqrt(HD).item())
                return at

            def scores_head(c, h):
                return [scores_group(c, h, grp) for grp in JG]

            def av_head_ib(c, h, at_tiles, ao2_tiles):
                """per-i-block AV for the tail chunk: the drain (rec+mul)
                of block ib overlaps the fill of ib+1."""
                off = (h % 2) * HD
                for ib in range(CH // 128):
                    ps_a = ps_mix.tile([128, HD + 1], F32, name="ps_ai",
                                       tag="mix")
                    for j in range(JT):
                        g, t = j // 2, j % 2
                        nc.tensor.matmul(
                            ps_a[:],
                            lhsT=at_tiles[g][:, t * CH + ib * 128:
                                             t * CH + (ib + 1) * 128],
                            rhs=v_aug[h][j // 4][:, j % 4, :],
                            start=(j == 0), stop=(j == JT - 1))
                    rec = small.tile([128, 1], F32, name="reci", tag="rec")
                    nc.vector.reciprocal(rec[:], ps_a[:, HD:HD + 1])
                    nc.vector.tensor_scalar_mul(
                        ao2_tiles[h // 2][ib][:, off:off + HD],
                        ps_a[:, 0:HD], rec[:])

            def av_head(c, h, at_tiles, ao2_tiles):
                """AV natural + per-partition normalize -> writes the head's
                64 columns of the pair tiles ao2 [128 i, 128].  All four
                i-blocks accumulate into one PSUM tile: one reciprocal and
                four muls per head instead of per block."""
                off = (h % 2) * HD
                ps_a = ps_mix.tile([128, 4, HD + 1], F32, name="ps_a", tag="mix")
                for ib in range(CH // 128):
                    for j in range(JT):
                        g, t = j // 2, j % 2
                        nc.tensor.matmul(
                            ps_a[:, ib, :],
                            lhsT=at_tiles[g][:, t * CH + ib * 128:
                                             t * CH + (ib + 1) * 128],
                            rhs=v_aug[h][j // 4][:, j % 4, :],
                            start=(j == 0), stop=(j == JT - 1))
                rec = small.tile([128, 4], F32, name="rec", tag="rec")
                nc.vector.reciprocal(rec[:], ps_a[:, :, HD])
                for ib in range(CH // 128):
                    nc.vector.tensor_scalar_mul(
                        ao2_tiles[h // 2][ib][:, off:off + HD],
                        ps_a[:, ib, 0:HD], rec[:, ib:ib + 1])

            def transpose_one(ao2, aoT2, p, ib):
                ps_t = ps_mix.tile([128, 128], F16, name="ps_t", tag="mix")
                nc.tensor.transpose(ps_t[:], ao2[p][ib][:], ident[:])
                t = aotpool.tile([128, 128], F16, name="aoT2", tag="aoT2")
                nc.vector.tensor_copy(out=t[:], in_=ps_t[:])
                aoT2[p][ib] = t

            def transpose_pair(ao2, aoT2, p):
                for ib in range(CH // 128):
                    transpose_one(ao2, aoT2, p, ib)

            def wo_ib(c, aoT2_tiles, ib, tail=False):
                i0 = c * CH
                osb = outpool.tile([128, DIM], F32, name="osb", tag="osb")
                for half in range(2):
                    ps_o = ps_mix.tile([128, 512], F32, name="ps_o",
                                        tag="mix")
                    for p in range(2):
                        nc.tensor.matmul(
                            ps_o[:],
                            lhsT=aoT2_tiles[p][ib][:],
                            rhs=wo_sb[p][:, half * 512:(half + 1) * 512],
                            start=(p == 0), stop=(p == 1))
                    if tail and half == 1:
                        # tail: ACT is done with exps -- copy halves in
                        # parallel on DVE and ACT
                        nc.scalar.activation(
                            out=osb[:, half * 512:(half + 1) * 512],
                            in_=ps_o[:],
                            func=mybir.ActivationFunctionType.Copy)
                    else:
                        nc.vector.tensor_copy(
                            out=osb[:, half * 512:(half + 1) * 512], in_=ps_o[:])
                    if tail:
                        # per-half DMA shortens the last-block chain
                        nc.sync.dma_start(
                            out=out_d[i0 + ib * 128:i0 + (ib + 1) * 128,
                                      half * 512:(half + 1) * 512],
                            in_=osb[:, half * 512:(half + 1) * 512])
                if not tail:
                    nc.sync.dma_start(
                        out=out_d[i0 + ib * 128:i0 + (ib + 1) * 128, :],
                        in_=osb[:])

            def new_ao2(c):
                return [[aopool.tile([128, 128], F16, name="ao2", tag="ao2")
                         for _ in range(CH // 128)] for _ in range(2)]

            # Emission = scheduler priority.  The Tile list-scheduler places
            # the highest-priority READY instruction whenever an engine
            # frees, so: projections first (deps of everything), then ALL
            # score groups in ACT consumption order (the exp stream is the
            # pacing engine), then V / AV / Wo LAST -- the scheduler floats
            # them into PE stalls (ps_s backpressure) automatically.
            NEWG = [[JG[2 * qi], JG[2 * qi + 1]] for qi in range(NQ)]
            at_all = [[[None] * len(JG) for _ in range(HL)]
                      for _ in range(NCH)]
            # K/Q0 with chunk-0 score groups streamed in (first exp ~9us)
            for n in range(2):
                for qi in range(NQ):
                    qk_proj_n((wk_lo, wk_hi), KTt, qi, n)
                    if qi == 0:
                        qk_proj_n(wq_sb, QT, 0, n)
                    for grp in NEWG[qi]:
                        for h in (2 * n, 2 * n + 1):
                            at_all[0][h][JG.index(grp)] = scores_group(0, h, grp)
                    # V batches ride in the PE slack while ACT drains exps
                    v_proj_hq(2 * n, qi)
                    v_proj_hq(2 * n + 1, qi)
            at_cur = at_all[0]
            pending_wo = None  # previous chunk's (c, aoT2): spread per-ib
            for c in range(NCH):
                ao2 = new_ao2(c)
                aoT2 = [[None] * (CH // 128) for _ in range(2)]
                at_next = None
                for h in range(HL):
                    # keep PE fed: next chunk's scores interleave with AV
                    if c + 1 < NCH:
                        if h == 0:
                            qk_proj_n(wq_sb, QT, c + 1, 0)
                        if h == 2:
                            qk_proj_n(wq_sb, QT, c + 1, 1)
                        if at_next is None:
                            at_next = []
                        at_next.append(scores_head(c + 1, h))
                    if pending_wo is not None:
                        wo_ib(pending_wo[0], pending_wo[1], h)
                    if c == NCH - 1 and h == HL - 1:
                        av_head_ib(c, h, at_cur[h], ao2)
                    else:
                        av_head(c, h, at_cur[h], ao2)
                    # transpose each head pair as soon as it completes so
                    # only pair 1 sits on the critical tail
                    if h == 1:
                        transpose_pair(ao2, aoT2, 0)
                at_cur = at_next
                if c < NCH - 1:
                    transpose_pair(ao2, aoT2, 1)
                    pending_wo = (c, aoT2)
                else:
                    # tail: transpose+Wo interleaved per i-block, copies
                    # split across DVE and the now-idle ACT
                    for ib in range(CH // 128):
                        transpose_one(ao2, aoT2, 1, ib)
                        wo_ib(c, aoT2, ib, tail=True)
    nc.compile()
    return nc


_NC_CACHE = None


def _get_nc():
    global _NC_CACHE
    if _NC_CACHE is None:
        _NC_CACHE = build_nc()
    return _NC_CACHE


def kernel(q, Wq, Wk, Wv, Wo, _trace=False, _results=None):
    q = np.asarray(q, np.float32)
    WqT = np.asarray(Wq, np.float32).T.astype(np.float16)
    WkT = np.asarray(Wk, np.float32).T.astype(np.float16)
    WvT = np.asarray(Wv, np.float32).T.astype(np.float16)
    WoT = np.asarray(Wo, np.float32).T.astype(np.float16)

    nc = _get_nc()
    in_maps = []
    for c in range(8):
        b, g = c // 4, c % 4
        hs = slice(DL * g, DL * (g + 1))
        in_maps.append({
            "xT": np.ascontiguousarray(q[b].T.astype(np.float16)),
            "wq": np.ascontiguousarray(WqT[:, hs]),
            "wk": np.ascontiguousarray(WkT[:, hs]),
            "wv": np.ascontiguousarray(WvT[:, hs]),
            "wo": np.ascontiguousarray(WoT[hs, :]),
        })
    res = run_bass_kernel_spmd(
        nc, in_maps, core_ids=list(range(8)), trace=_trace)
    if _results is not None:
        _results.append(res)
    out = np.empty((B, L, DIM), np.float32)
    for b in range(B):
        acc = res.results[4 * b]["out"].astype(np.float32)
        for g in range(1, 4):
            acc = acc + res.results[4 * b + g]["out"]
        out[b] = acc
    return out


# revision 41
# speedup vs baseline: 1.0198x; 1.0198x over previous
"""MultiHeadAttention TRN2 kernel: B=2, L=2048, DIM=1024, 16 heads x 64.

Sharding: 8 cores = 2 (batch) x 4 (head groups of 4 heads), tensor-parallel
on heads (Wq/Wk/Wv column-split) with Wo ROW-split: each core computes a
full-width partial output out_partial[L, DIM] from its 4 heads; the host
sums the 4 partials per batch during unshard (the all-reduce of the
row-split Wo, performed at gather time).  No device collectives.

Per core (all matmul operands fp16, PSUM accumulation fp32):
  - xT16 = q[b].T [1024, 2048], wq/wk/wv = W.T[:, headslice] [1024, 256],
    wo = Wo.T[headslice, :] [256, 1024] -- all fp16, host-converted.
  - K projected first (transposed layout KT [d, j]), then Q chunk-0, so
    scores/exp for chunk 0 start ~20us in; V (natural [j, h, 64+1] with a
    ones column -> softmax denominator) and remaining Q quarters follow.
  - scores per (chunk, head): j-tile pairs share one PSUM tile so one
    ACTIVATE exps 1024 elems; exp(0.125*s) -> fp16 attn tiles [j, i].
  - AV in NATURAL orientation (lhsT=attnT, rhs=[v|1]): out [i-block, 65]
    -- full 128-row stationary vs 65 in the transposed form (2x fewer PE
    rows), denominator lands in column 64 per i-PARTITION, so the
    normalize is a per-partition tensor_scalar (no PE broadcast matmul).
  - head pairs packed side by side [i, 128], DMA-TRANSPOSED (xbar, off-PE)
    into the Wo lhsT layout [2x64 d, i].
  - Wo partial: out[i, 1024] = sum over 2 d-pair k-tiles; PSUM -> SBUF f32
    copy on gpsimd (Pool), DMA out.  Host sums group partials.
Emission interleaves scores(c+1, h) ahead of AV(c, h) per head so the PE
stays busy while ACT (the #2 engine, ~128us of exp) catches up.
"""

import sys
from contextlib import ExitStack

import numpy as np

for _p in ("/opt/trn_rl_repo",):
    if _p not in sys.path:
        sys.path.insert(0, _p)

import concourse.bass as bass
import concourse.tile as tile
from concourse import bacc, masks, mybir
from concourse.bass_utils import run_bass_kernel_spmd

F32 = mybir.dt.float32
F16 = mybir.dt.float16

B, L, DIM = 2, 2048, 1024
NH, HD = 16, 64           # total heads, head dim
HL = 4                    # heads per core
DL = HL * HD              # local head dims = 256
KT = DIM // 128           # 8  contraction k-tiles
JT = L // 128             # 16 j (key) tiles
CH = 512                  # i-chunk size
NCH = L // CH             # 4 chunks
NQ = 4                    # L quarters for projection streaming
QLF = L // NQ             # 512


def build_nc():
    nc = bacc.Bacc("TRN2", target_bir_lowering=False, debug=False, num_devices=8)

    xT_d = nc.dram_tensor("xT", [DIM, L], F16, kind="ExternalInput")
    wq_d = nc.dram_tensor("wq", [DIM, DL], F16, kind="ExternalInput")
    wk_d = nc.dram_tensor("wk", [DIM, DL], F16, kind="ExternalInput")
    wv_d = nc.dram_tensor("wv", [DIM, DL], F16, kind="ExternalInput")
    wo_d = nc.dram_tensor("wo", [DL, DIM], F16, kind="ExternalInput")
    out_d = nc.dram_tensor("out", [L, DIM], F32, kind="ExternalOutput")

    with tile.TileContext(nc) as tc:
        with ExitStack() as ctx:
            wpool = ctx.enter_context(tc.tile_pool(name="weights", bufs=3))
            wopool = ctx.enter_context(tc.tile_pool(name="wo", bufs=2))
            xpool = ctx.enter_context(tc.tile_pool(name="xT", bufs=8))
            xqpool = ctx.enter_context(tc.tile_pool(name="xTq", bufs=3))
            qkpool = ctx.enter_context(tc.tile_pool(name="qk", bufs=16))
            vpool = ctx.enter_context(tc.tile_pool(name="v", bufs=16))
            atpool = ctx.enter_context(tc.tile_pool(name="attnT", bufs=40))
            aopool = ctx.enter_context(tc.tile_pool(name="ao", bufs=10))
            aotpool = ctx.enter_context(tc.tile_pool(name="aot", bufs=18))
            small = ctx.enter_context(tc.tile_pool(name="small", bufs=6))
            outpool = ctx.enter_context(tc.tile_pool(name="outsb", bufs=4))
            # ps_s: 3 slots of [128, 1024] f32 (2 banks each): ~2.7us of
            # ACT lookahead with fine production granularity.
            ps_s = ctx.enter_context(
                tc.tile_pool(name="ps_s", bufs=3, space="PSUM"))
            # ps_mix: everything else (proj [128,512], AV [128,65],
            # PE-transpose [128,128]f16, Wo [128,512]) in 2 one-bank slots
            ps_mix = ctx.enter_context(
                tc.tile_pool(name="ps_mix", bufs=2, space="PSUM"))

            # ---- weights (as [128, KT, DL] k-tile stacks) ----
            # HWDGE (SP/ACT-issued DMA) is an exclusive device with a 625ns
            # hold per DMA: split loads between SP (HWDGE) and gpsimd (SWDGE,
            # desc-gen on the otherwise idle Pool engine) so neither serializes
            # the projection pipeline.
            def load_w(dram_t, name, eng):
                t = wpool.tile([128, KT, DL], F16, name=name, tag="w")
                eng.dma_start(
                    out=t[:], in_=dram_t[:].rearrange("(k p) n -> p k n", p=128))
                return t

            # wk split in k-halves across SP (HWDGE) and gpsimd (SWDGE):
            # the first half lands ~1.3us earlier, so K proj starts sooner
            wk_view = wk_d[:].rearrange("(k p) n -> p k n", p=128)
            wk_lo = wpool.tile([128, KT // 2, DL], F16, name="wk_lo", tag="wkh")
            nc.sync.dma_start(out=wk_lo[:], in_=wk_view[:, 0:KT // 2, :])
            wk_hi = wpool.tile([128, KT // 2, DL], F16, name="wk_hi", tag="wkh")
            nc.gpsimd.dma_start(out=wk_hi[:], in_=wk_view[:, KT // 2:KT, :])

            # xT loads, all SP/HWDGE: quarter 0 split per-k (first tile lands
            # ~2us so K proj starts immediately); quarters 1-3 as ONE big DMA
            # each (one 625ns HWDGE hold instead of eight)
            xT_view = xT_d[:].rearrange("(k p) n -> p k n", p=128)
            xT_k = [[None] * KT for _ in range(NQ)]
            for k in range(KT):
                xt = xpool.tile([128, QLF], F16, name="xt0", tag="xt")
                nc.sync.dma_start(out=xt[:], in_=xT_view[:, k, 0:QLF])
                xT_k[0][k] = xt
            wq_sb = load_w(wq_d, "wq_sb", nc.sync)
            for qi in range(1, NQ):
                xt = xqpool.tile([128, KT, QLF], F16, name="xtq", tag="xtq")
                nc.sync.dma_start(
                    out=xt[:], in_=xT_view[:, :, qi * QLF:(qi + 1) * QLF])
                for k in range(KT):
                    xT_k[qi][k] = xt[:, k, :]
            wv_sb = load_w(wv_d, "wv_sb", nc.gpsimd)
            # wo natural [256, 1024] -> 2 k-tiles [128, 1024]
            wo_view = wo_d[:].rearrange("(t p) n -> t p n", p=128)
            wo_sb = []
            for t in range(2):
                w = wopool.tile([128, DIM], F16, name=f"wo{t}", tag="wo")
                nc.gpsimd.dma_start(out=w[:], in_=wo_view[t])
                wo_sb.append(w)

            # fp16 identity for PE-transpose (53ns/tile vs 625ns HWDGE hold
            # for the xbar DMA transpose)
            ident = small.tile([128, 128], F16, name="ident", tag="ident")
            masks.make_identity(nc, ident[:])

            # V natural, one tile per (head, j-quarter): [128, 4, 65]
            # (col 64 = ones -> denominator).  Per-head tiles let the V
            # projection stream per head inside chunk 0's loop; batching 4
            # j-tiles per PSUM drain keeps the fill/drain ratio healthy.
            v_aug = [[vpool.tile([128, 4, HD + 1], F16, name="va", tag="va")
                      for _ in range(NQ)] for _ in range(HL)]
            for row in v_aug:
                for va in row:
                    nc.vector.memset(va[:, :, HD:HD + 1], 1.0)

            # ---- projections ----
            # per-quarter tiles keep Tile deps fine-grained
            QT = [[None] * NQ for _ in range(2)]
            KTt = [[None] * NQ for _ in range(2)]

            def qk_proj_n(w_sb, tiles, qi, n):
                ps = ps_mix.tile([128, QLF], F32, name="ps_p", tag="mix")
                for k in range(KT):
                    if isinstance(w_sb, tuple):
                        w = w_sb[k // (KT // 2)]
                        wsl = w[:, k % (KT // 2), n * 128:(n + 1) * 128]
                    else:
                        wsl = w_sb[:, k, n * 128:(n + 1) * 128]
                    nc.tensor.matmul(
                        ps[:], lhsT=wsl, rhs=xT_k[qi][k][:],
                        start=(k == 0), stop=(k == KT - 1))
                t = qkpool.tile([128, QLF], F16, name="qkt", tag="qkt")
                nc.vector.tensor_copy(out=t[:], in_=ps[:])
                tiles[n][qi] = t

            def qk_proj_quarter(w_sb, tiles, qi):
                for n in range(2):
                    qk_proj_n(w_sb, tiles, qi, n)

            def v_proj_hq(h, qi):
                ps = ps_mix.tile([128, 4, HD], F32, name="ps_v", tag="mix")
                for m in range(4):
                    for k in range(KT):
                        nc.tensor.matmul(
                            ps[:, m, :],
                            lhsT=xT_k[qi][k][:, m * 128:(m + 1) * 128],
                            rhs=wv_sb[:, k, h * HD:(h + 1) * HD],
                            start=(k == 0), stop=(k == KT - 1))
                nc.vector.tensor_copy(
                    out=v_aug[h][qi][:, :, 0:HD], in_=ps[:])

            # ---- attention ----
            # j-tile pairs: one [128, 1024] exp per pair
            JG = [(2 * g, 2 * g + 1) for g in range(JT // 2)]

            def scores_group(c, h, grp):
                """one j-group's scores + exp -> fp16 attn tile
                [128 j, len(grp)*CH]."""
                ht, hr = h // 2, 64 * (h % 2)
                w = len(grp) * CH
                ps_sc = ps_s.tile([128, w], F32, name="ps_sc", tag="ps_s")
                for s, j in enumerate(grp):
                    nc.tensor.matmul(
                        ps_sc[:, s * CH:(s + 1) * CH],
                        lhsT=KTt[ht][j // 4][hr:hr + 64,
                                             (j % 4) * 128:(j % 4 + 1) * 128],
                        rhs=QT[ht][c][hr:hr + 64, :],
                        start=True, stop=True)
                at = atpool.tile([128, w], F16, name="at", tag="at")
                nc.scalar.activation(
                    out=at[:], in_=ps_sc[:],
                    func=mybir.ActivationFunctionType.Exp,
                    scale=1.0 / np.sqrt(HD).item())
                return at

            def scores_head(c, h):
                return [scores_group(c, h, grp) for grp in JG]

            def av_head_ib(c, h, at_tiles, ao2_tiles):
                """per-i-block AV for the tail chunk: the drain (rec+mul)
                of block ib overlaps the fill of ib+1."""
                off = (h % 2) * HD
                for ib in range(CH // 128):
                    ps_a = ps_mix.tile([128, HD + 1], F32, name="ps_ai",
                                       tag="mix")
                    for j in range(JT):
                        g, t = j // 2, j % 2
                        nc.tensor.matmul(
                            ps_a[:],
                            lhsT=at_tiles[g][:, t * CH + ib * 128:
                                             t * CH + (ib + 1) * 128],
                            rhs=v_aug[h][j // 4][:, j % 4, :],
                            start=(j == 0), stop=(j == JT - 1))
                    rec = small.tile([128, 1], F32, name="reci", tag="rec")
                    nc.vector.reciprocal(rec[:], ps_a[:, HD:HD + 1])
                    nc.vector.tensor_scalar_mul(
                        ao2_tiles[h // 2][ib][:, off:off + HD],
                        ps_a[:, 0:HD], rec[:])

            def av_head(c, h, at_tiles, ao2_tiles):
                """AV natural + per-partition normalize -> writes the head's
                64 columns of the pair tiles ao2 [128 i, 128].  All four
                i-blocks accumulate into one PSUM tile: one reciprocal and
                four muls per head instead of per block."""
                off = (h % 2) * HD
                ps_a = ps_mix.tile([128, 4, HD + 1], F32, name="ps_a", tag="mix")
                for ib in range(CH // 128):
                    for j in range(JT):
                        g, t = j // 2, j % 2
                        nc.tensor.matmul(
                            ps_a[:, ib, :],
                            lhsT=at_tiles[g][:, t * CH + ib * 128:
                                             t * CH + (ib + 1) * 128],
                            rhs=v_aug[h][j // 4][:, j % 4, :],
                            start=(j == 0), stop=(j == JT - 1))
                rec = small.tile([128, 4], F32, name="rec", tag="rec")
                nc.vector.reciprocal(rec[:], ps_a[:, :, HD])
                for ib in range(CH // 128):
                    nc.vector.tensor_scalar_mul(
                        ao2_tiles[h // 2][ib][:, off:off + HD],
                        ps_a[:, ib, 0:HD], rec[:, ib:ib + 1])

            def transpose_one(ao2, aoT2, p, ib):
                ps_t = ps_mix.tile([128, 128], F16, name="ps_t", tag="mix")
                nc.tensor.transpose(ps_t[:], ao2[p][ib][:], ident[:])
                t = aotpool.tile([128, 128], F16, name="aoT2", tag="aoT2")
                nc.vector.tensor_copy(out=t[:], in_=ps_t[:])
                aoT2[p][ib] = t

            def transpose_pair(ao2, aoT2, p):
                for ib in range(CH // 128):
                    transpose_one(ao2, aoT2, p, ib)

            def wo_ib(c, aoT2_tiles, ib, tail=False):
                i0 = c * CH
                osb = outpool.tile([128, DIM], F32, name="osb", tag="osb")
                for half in range(2):
                    ps_o = ps_mix.tile([128, 512], F32, name="ps_o",
                                        tag="mix")
                    for p in range(2):
                        nc.tensor.matmul(
                            ps_o[:],
                            lhsT=aoT2_tiles[p][ib][:],
                            rhs=wo_sb[p][:, half * 512:(half + 1) * 512],
                            start=(p == 0), stop=(p == 1))
                    if tail and half == 1:
                        # tail: ACT is done with exps -- copy halves in
                        # parallel on DVE and ACT
                        nc.scalar.activation(
                            out=osb[:, half * 512:(half + 1) * 512],
                            in_=ps_o[:],
                            func=mybir.ActivationFunctionType.Copy)
                    else:
                        nc.vector.tensor_copy(
                            out=osb[:, half * 512:(half + 1) * 512], in_=ps_o[:])
                    if tail:
                        # per-half DMA shortens the last-block chain
                        nc.sync.dma_start(
                            out=out_d[i0 + ib * 128:i0 + (ib + 1) * 128,
                                      half * 512:(half + 1) * 512],
                            in_=osb[:, half * 512:(half + 1) * 512])
                if not tail:
                    nc.sync.dma_start(
                        out=out_d[i0 + ib * 128:i0 + (ib + 1) * 128, :],
                        in_=osb[:])

            def new_ao2(c):
                return [[aopool.tile([128, 128], F16, name="ao2", tag="ao2")
                         for _ in range(CH // 128)] for _ in range(2)]

            # Emission = scheduler priority.  The Tile list-scheduler places
            # the highest-priority READY instruction whenever an engine
            # frees, so: projections first (deps of everything), then ALL
            # score groups in ACT consumption order (the exp stream is the
            # pacing engine), then V / AV / Wo LAST -- the scheduler floats
            # them into PE stalls (ps_s backpressure) automatically.
            NEWG = [[JG[2 * qi], JG[2 * qi + 1]] for qi in range(NQ)]
            at_all = [[[None] * len(JG) for _ in range(HL)]
                      for _ in range(NCH)]
            # K/Q0 with chunk-0 score groups streamed in (first exp ~9us)
            for n in range(2):
                for qi in range(NQ):
                    qk_proj_n((wk_lo, wk_hi), KTt, qi, n)
                    if qi == 0:
                        qk_proj_n(wq_sb, QT, 0, n)
                    for grp in NEWG[qi]:
                        for h in (2 * n, 2 * n + 1):
                            at_all[0][h][JG.index(grp)] = scores_group(0, h, grp)
                    # V batches ride in the PE slack while ACT drains exps
                    v_proj_hq(2 * n, qi)
                    v_proj_hq(2 * n + 1, qi)
            at_cur = at_all[0]
            pending_wo = None  # previous chunk's (c, aoT2): spread per-ib
            for c in range(NCH):
                ao2 = new_ao2(c)
                aoT2 = [[None] * (CH // 128) for _ in range(2)]
                at_next = None
                for h in range(HL):
                    # keep PE fed: next chunk's scores interleave with AV
                    if c + 1 < NCH:
                        if h == 0:
                            qk_proj_n(wq_sb, QT, c + 1, 0)
                        if h == 2:
                            qk_proj_n(wq_sb, QT, c + 1, 1)
                        if at_next is None:
                            at_next = []
                        at_next.append(scores_head(c + 1, h))
                    if pending_wo is not None:
                        wo_ib(pending_wo[0], pending_wo[1], h)
                    if c == NCH - 1 and h == HL - 1:
                        av_head_ib(c, h, at_cur[h], ao2)
                    else:
                        av_head(c, h, at_cur[h], ao2)
                    # transpose each head pair as soon as it completes so
                    # only pair 1 sits on the critical tail
                    if h == 1:
                        transpose_pair(ao2, aoT2, 0)
                at_cur = at_next
                if c < NCH - 1:
                    transpose_pair(ao2, aoT2, 1)
                    pending_wo = (c, aoT2)
                else:
                    # tail: transpose+Wo interleaved per i-block, copies
                    # split across DVE and the now-idle ACT
                    for ib in range(CH // 128):
                        transpose_one(ao2, aoT2, 1, ib)
                        wo_ib(c, aoT2, ib, tail=True)
    nc.compile()
    return nc


_NC_CACHE = None


def _get_nc():
    global _NC_CACHE
    if _NC_CACHE is None:
        _NC_CACHE = build_nc()
    return _NC_CACHE


def kernel(q, Wq, Wk, Wv, Wo, _trace=False, _results=None):
    q = np.asarray(q, np.float32)
    WqT = np.asarray(Wq, np.float32).T.astype(np.float16)
    WkT = np.asarray(Wk, np.float32).T.astype(np.float16)
    WvT = np.asarray(Wv, np.float32).T.astype(np.float16)
    WoT = np.asarray(Wo, np.float32).T.astype(np.float16)

    nc = _get_nc()
    in_maps = []
    for c in range(8):
        b, g = c // 4, c % 4
        hs = slice(DL * g, DL * (g + 1))
        in_maps.append({
            "xT": np.ascontiguousarray(q[b].T.astype(np.float16)),
            "wq": np.ascontiguousarray(WqT[:, hs]),
            "wk": np.ascontiguousarray(WkT[:, hs]),
            "wv": np.ascontiguousarray(WvT[:, hs]),
            "wo": np.ascontiguousarray(WoT[hs, :]),
        })
    res = run_bass_kernel_spmd(
        nc, in_maps, core_ids=list(range(8)), trace=_trace)
    if _results is not None:
        _results.append(res)
    out = np.empty((B, L, DIM), np.float32)
    for b in range(B):
        acc = res.results[4 * b]["out"].astype(np.float32)
        for g in range(1, 4):
            acc = acc + res.results[4 * b + g]["out"]
        out[b] = acc
    return out
